# revision 1
# baseline (speedup 1.0000x reference)
"""Mamba mixer (nn_Mixer) Trainium2 Bass kernel, v2.

Sharding: tensor-parallel over d_inner (2048 -> 256 per core, 8 cores).

Two-pass structure per core:
  Pass A (per 512-token chunk): in_proj (bf16), causal conv1d + silu,
    silu(z), x_proj partial -> DRAM.  All ACT ops are Silu/Copy.
  AllReduce: n_ar collectives over the full-sequence x_dbl partials
    (instead of one per chunk on the critical path).
  Pass B (per chunk): dt softplus (Exp/Ln), selective scan
    (tensor_tensor_scan) with B/C broadcast via PE selector matmuls,
    gating, out_proj partial -> DRAM (f32, host sums across cores).

Self-contained: hardcodes all shapes.
"""

import os
import numpy as np

D_MODEL = 1024
D_INNER = 2048
NSTATE = 16
DT_RANK = 64
DCONV = 4
BATCH = 2
SEQ = 4096

NCORES = 8
DS = D_INNER // NCORES          # 256 d_inner rows per core
DT2 = DS // 128                 # 2 partition tiles per core
TOK = BATCH * SEQ


def _build_nc(lc=512, fake_collective=False, n_ar=2, lag=5,
              bc_act=16, mul_pool=8, ob_act=6, ladder=False,
              conv_pool=False, psy2=False, small_pool=True,
              skip_ar=False, passes="ab", scan_as_mul=False):
    """Build the Bass program (same SPMD program for all 8 cores)."""
    import concourse.bass as bass
    import concourse.bacc as bacc
    import concourse.mybir as mybir
    import concourse.tile as tile

    f32 = mybir.dt.float32
    f32r = mybir.dt.float32r
    bf16 = mybir.dt.bfloat16
    AF = mybir.ActivationFunctionType
    OP = mybir.AluOpType

    n_chunks = TOK // lc
    chunks_per_b = SEQ // lc
    chunks_per_ar = n_chunks // n_ar
    # pass-B chunk c reads ar_out[seg(c)], whose collective is emitted after
    # pass-A chunk (seg+1)*chunks_per_ar-1; a smaller lag would make the
    # read precede the write in program order (= reads uninitialized DRAM).
    lag = max(lag, chunks_per_ar)

    nc = bacc.Bacc("TRN2", target_bir_lowering=False, debug=False,
                   num_devices=NCORES)

    # ---- kernel I/O (per-core shards prepared on the host) ----
    uT = nc.dram_tensor("uT", [D_MODEL, TOK], bf16, kind="ExternalInput")
    w_in = nc.dram_tensor("w_inT", [D_MODEL, 4 * 128], bf16, kind="ExternalInput")
    conv_w = nc.dram_tensor("conv_w", [128, DT2 * DCONV], f32, kind="ExternalInput")
    conv_b = nc.dram_tensor("conv_b", [128, DT2], f32, kind="ExternalInput")
    w_xp = nc.dram_tensor("w_xpT", [DS, DT_RANK + 2 * NSTATE], bf16, kind="ExternalInput")
    w_dt = nc.dram_tensor("w_dtT", [DT_RANK, DS], bf16, kind="ExternalInput")
    dt_bias = nc.dram_tensor("dt_bias", [128, DT2], f32, kind="ExternalInput")
    a_neg = nc.dram_tensor("a_neg", [128, DT2 * NSTATE], f32, kind="ExternalInput")
    d_in = nc.dram_tensor("d_in", [128, DT2], f32, kind="ExternalInput")
    w_out = nc.dram_tensor("w_outT", [DS, D_MODEL], bf16, kind="ExternalInput")
    eye_d = nc.dram_tensor("eye128", [128, 128], bf16, kind="ExternalInput")
    sel_d = nc.dram_tensor("sel32", [2 * NSTATE, 2 * NSTATE * 128], bf16,
                           kind="ExternalInput")
    y_part = nc.dram_tensor("y_part", [D_MODEL, TOK], bf16, kind="ExternalOutput")

    NXD = DT_RANK + 2 * NSTATE  # 96

    with tile.TileContext(nc) as tc:
        with (
            tc.tile_pool(name="const", bufs=1) as cpool,
            tc.tile_pool(name="u", bufs=2) as upool,
            tc.tile_pool(name="seq", bufs=1) as qpool,      # full-seq xs/z
            tc.tile_pool(name="work", bufs=2) as wpool,
            tc.tile_pool(name="nwork", bufs=5) as npool,
            tc.tile_pool(name="small", bufs=3) as spool,
            tc.tile_pool(name="mm", bufs=2, space="PSUM") as psmm,
            tc.tile_pool(name="psbc", bufs=(1 if psy2 else 2),
                         space="PSUM") as psbc,
            tc.tile_pool(name="psy", bufs=(2 if psy2 else 1),
                         space="PSUM") as psy,
            tc.tile_pool(name="dram", bufs=1, space="DRAM") as dpool,
        ):
            # ---- static weights into SBUF ----
            w_in_sb = cpool.tile([128, 8, 4 * 128], bf16)
            nc.sync.dma_start(w_in_sb[:], w_in.ap().rearrange(
                "(j p) m -> p j m", p=128))
            w_out_sb = cpool.tile([128, DT2, D_MODEL], bf16)
            nc.sync.dma_start(w_out_sb[:], w_out.ap().rearrange(
                "(k p) m -> p k m", p=128))
            w_xp_sb = cpool.tile([128, DT2, NXD], bf16)
            nc.sync.dma_start(w_xp_sb[:], w_xp.ap().rearrange(
                "(k p) m -> p k m", p=128))
            w_dt_sb = cpool.tile([DT_RANK, DS], bf16)
            nc.sync.dma_start(w_dt_sb[:], w_dt.ap())
            conv_w_sb = cpool.tile([128, DT2 * DCONV], f32)
            nc.sync.dma_start(conv_w_sb[:], conv_w.ap())
            conv_b_sb = cpool.tile([128, DT2], f32)
            nc.sync.dma_start(conv_b_sb[:], conv_b.ap())
            dt_bias_sb = cpool.tile([128, DT2], f32)
            nc.sync.dma_start(dt_bias_sb[:], dt_bias.ap())
            a_sb = cpool.tile([128, DT2 * NSTATE], f32)
            nc.sync.dma_start(a_sb[:], a_neg.ap())
            d_in_sb = cpool.tile([128, DT2], f32)
            nc.sync.dma_start(d_in_sb[:], d_in.ap())
            eye16 = cpool.tile([128, 128], bf16)
            nc.sync.dma_start(eye16[:], eye_d.ap())
            sel_sb = cpool.tile([2 * NSTATE, 2 * NSTATE * 128], bf16)
            nc.sync.dma_start(sel_sb[:], sel_d.ap())

            # full-sequence conv+silu output and silu(z), SBUF-resident
            xs_all = qpool.tile([128, DT2, TOK], bf16)
            z_all = qpool.tile([128, DT2, TOK], bf16)
            # scan state carried across chunks: one column per (dtile, n)
            carry = cpool.tile([128, DT2 * NSTATE], f32)

            # AllReduce segments (DRAM)
            seg_tok = TOK // n_ar
            ar_in = [dpool.tile([NXD, seg_tok], bf16, name=f"arin{s}",
                                tag=f"arin{s}")
                     for s in range(n_ar)]
            ar_out = [dpool.tile([NXD, seg_tok], bf16, name=f"arout{s}",
                                 tag=f"arout{s}", addr_space="Shared")
                      for s in range(n_ar)]

            uT_ap = uT.ap().rearrange("(j p) t -> p j t", p=128)

            # Bresenham-style static op assignment counters
            cnt = {}

            def pick2(key, frac_num=10, frac_den=16):
                # returns 0 for the first engine frac_num/frac_den of the time
                c = cnt.get(key, 0)
                cnt[key] = c + 1
                return 0 if (c * frac_num) % frac_den < frac_num else 1

            bres = {"mul": 0}

            def pickmul():
                # 20 of 32 mul sites -> pool, rest dve
                c = bres["mul"]
                bres["mul"] = c + 1
                return "pool" if (c * mul_pool % 32) < mul_pool else "dve"

            def mul_to(eng, out, in0, in1):
                (nc.vector if eng == "dve" else nc.gpsimd).tensor_mul(
                    out, in0, in1)

            # ================= Pass A =================
            prev_x = [None]

            def emit_pass_a(c):
                t0 = c * lc
                first_in_batch = (c % chunks_per_b) == 0
                u_sb = upool.tile([128, 8, lc], bf16, tag="u")
                nc.sync.dma_start(u_sb[:], uT_ap[:, :, t0:t0 + lc])

                x_sb = wpool.tile([128, DT2, lc + DCONV - 1], bf16, tag="x")
                for mt in range(4):
                    ps = psmm.tile([128, lc], f32, tag="mm")
                    for j in range(8):
                        nc.tensor.matmul(
                            ps[:],
                            w_in_sb[:, j, 128 * mt:128 * (mt + 1)],
                            u_sb[:, j, :],
                            start=(j == 0), stop=(j == 7))
                    if mt < DT2:
                        nc.scalar.copy(x_sb[:, mt, DCONV - 1:], ps[:])
                    else:
                        nc.scalar.activation(z_all[:, mt - DT2, t0:t0 + lc],
                                             ps[:], AF.Silu, bias=0.0)

                # halo columns (previous chunk's last DCONV-1 raw x)
                for dt in range(DT2):
                    if first_in_batch:
                        nc.vector.memset(x_sb[:, dt, 0:DCONV - 1], 0.0)
                    else:
                        nc.vector.tensor_copy(
                            x_sb[:, dt, 0:DCONV - 1],
                            prev_x[0][:, dt, lc:lc + DCONV - 1])
                prev_x[0] = x_sb

                # causal conv1d + silu -> xs_all
                for dt in range(DT2):
                    ca = spool.tile([128, lc], bf16, tag="ca")
                    cb = spool.tile([128, lc], bf16, tag="cb")
                    if conv_pool:
                        # Pool-legal form: TT-mult with broadcast w column,
                        # then TT-adds (gpsimd has no tensor_scalar/stt)
                        tp = spool.tile([128, DCONV - 1, lc], bf16, tag="tp")
                        for k in range(1, DCONV):
                            nc.gpsimd.tensor_mul(
                                tp[:, k - 1, :], x_sb[:, dt, k:k + lc],
                                conv_w_sb[:, dt * DCONV + k:
                                          dt * DCONV + k + 1]
                                .broadcast_to([128, lc]))
                        nc.vector.tensor_scalar_mul(
                            ca[:], x_sb[:, dt, 0:lc],
                            conv_w_sb[:, dt * DCONV:dt * DCONV + 1])
                        nc.gpsimd.tensor_add(cb[:], ca[:], tp[:, 0, :])
                        nc.gpsimd.tensor_add(ca[:], tp[:, 1, :], tp[:, 2, :])
                        nc.gpsimd.tensor_add(cb[:], ca[:], cb[:])
                        srcv = cb
                    else:
                        nc.vector.tensor_scalar_mul(
                            ca[:], x_sb[:, dt, 0:lc],
                            conv_w_sb[:, dt * DCONV:dt * DCONV + 1])
                        srcv, dst = ca, cb
                        for k in range(1, DCONV):
                            nc.vector.scalar_tensor_tensor(
                                dst[:], x_sb[:, dt, k:k + lc],
                                conv_w_sb[:, dt * DCONV + k:
                                          dt * DCONV + k + 1],
                                srcv[:], op0=OP.mult, op1=OP.add)
                            srcv, dst = dst, srcv
                    nc.scalar.activation(xs_all[:, dt, t0:t0 + lc], srcv[:],
                                         AF.Silu,
                                         bias=conv_b_sb[:, dt:dt + 1])

                # x_proj partial -> DRAM segment
                ps_xd = psmm.tile([NXD, lc], f32, tag="mm")
                for dt in range(DT2):
                    nc.tensor.matmul(ps_xd[:], w_xp_sb[:, dt, :],
                                     xs_all[:, dt, t0:t0 + lc],
                                     start=(dt == 0), stop=(dt == DT2 - 1))
                xd_loc = spool.tile([NXD, lc], bf16, tag="xdloc")
                nc.scalar.copy(xd_loc[:], ps_xd[:])
                seg = c // chunks_per_ar
                s0 = t0 - seg * seg_tok
                nc.sync.dma_start(ar_in[seg][:, s0:s0 + lc], xd_loc[:])

                # issue the collective as soon as its last chunk is in
                if (c + 1) % chunks_per_ar == 0:
                    if skip_ar:
                        pass
                    elif fake_collective:
                        nc.sync.dma_start(ar_out[seg][:], ar_in[seg][:])
                    else:
                        nc.gpsimd.collective_compute(
                            "AllReduce", OP.add,
                            replica_groups=[list(range(NCORES))],
                            ins=[ar_in[seg].opt()], outs=[ar_out[seg].opt()])

            # ================= Pass B =================
            # Software-pipelined: chunk c+1's dt-stage (xd DMA, dt_proj,
            # softplus, dtx) is emitted BEFORE chunk c's n-loop so the
            # in-order ACT queue never serializes consecutive chunks.
            def emit_dt_stage(c):
                t0 = c * lc
                seg = c // chunks_per_ar
                s0 = t0 - seg * seg_tok
                xdt_sb = spool.tile([DT_RANK, lc], bf16, tag="xdt")
                nc.sync.dma_start(xdt_sb[:],
                                  ar_out[seg][0:DT_RANK, s0:s0 + lc])
                bc32 = spool.tile([2 * NSTATE, lc], bf16, tag="bc32")
                nc.sync.dma_start(bc32[:],
                                  ar_out[seg][DT_RANK:NXD, s0:s0 + lc])

                # dt = softplus(dt_proj @ x_dbl[:64] + bias), via Exp/Ln
                dt_sb = wpool.tile([128, DT2, lc], bf16, tag="dt")
                for dt in range(DT2):
                    ps = psmm.tile([128, lc], f32, tag="mm")
                    nc.tensor.matmul(
                        ps[:], w_dt_sb[:, 128 * dt:128 * (dt + 1)],
                        xdt_sb[:], start=True, stop=True)
                    e_t = spool.tile([128, lc], f32, tag="esp")
                    nc.scalar.activation(e_t[:], ps[:], AF.Exp,
                                         bias=dt_bias_sb[:, dt:dt + 1])
                    nc.scalar.activation(dt_sb[:, dt, :], e_t[:], AF.Ln,
                                         bias=1.0)

                dtx_sb = wpool.tile([128, DT2, lc], bf16, tag="dtx")
                dtx_eng = nc.gpsimd if small_pool else nc.vector
                for dt in range(DT2):
                    dtx_eng.tensor_mul(dtx_sb[:, dt, :], dt_sb[:, dt, :],
                                       xs_all[:, dt, t0:t0 + lc])
                return c, t0, dt_sb, dtx_sb, bc32

            def emit_scan_loop(stage):
                c, t0, dt_sb, dtx_sb, bc32 = stage
                if (c % chunks_per_b) == 0:
                    nc.vector.memset(carry[:], 0.0)
                a_low = {}
                y_ps = psy.tile([128, DT2, lc], f32, tag="y")
                for n in range(NSTATE):
                    bc_ps = psbc.tile([128, 2, lc], f32, tag="bcps")
                    nc.tensor.matmul(
                        bc_ps[:, 0, :],
                        sel_sb[:, 128 * n:128 * (n + 1)],
                        bc32[:], start=True, stop=True)
                    nc.tensor.matmul(
                        bc_ps[:, 1, :],
                        sel_sb[:, 128 * (NSTATE + n):
                               128 * (NSTATE + n + 1)],
                        bc32[:], start=True, stop=True)
                    # copy B,C rows to bf16 SBUF (enables DVE 2x / Pool muls)
                    bc2 = npool.tile([128, 2, lc], bf16, tag="bc2")
                    if pick2("bc2", bc_act, 16) == 0:
                        nc.scalar.copy(bc2[:], bc_ps[:])
                    else:
                        nc.vector.tensor_copy(bc2[:], bc_ps[:])
                    bb = bc2[:, 0:1, :]
                    cc = bc2[:, 1:2, :]

                    a_both = npool.tile([128, DT2, lc], bf16, tag="a",
                                        bufs=10)
                    if ladder and n >= 8:
                        eng = "dve" if pick2("lad", 4, 8) == 0 else "pool"
                        mul_to(eng, a_both[:], a_low[n - 8][:],
                               a_low[7][:])
                    else:
                        for dt in range(DT2):
                            col = dt * NSTATE + n
                            nc.scalar.activation(a_both[:, dt, :],
                                                 dt_sb[:, dt, :], AF.Exp,
                                                 bias=0.0,
                                                 scale=a_sb[:, col:col + 1])
                    if ladder and n < 8:
                        a_low[n] = a_both
                    dbx = npool.tile([128, DT2, lc], bf16, tag="dbx")
                    mul_to(pickmul(), dbx[:],
                           dtx_sb[:], bb.broadcast_to([128, DT2, lc]))
                    h_t = npool.tile([128, DT2, lc], bf16, tag="h")
                    for dt in range(DT2):
                        col = dt * NSTATE + n
                        if scan_as_mul:
                            nc.vector.tensor_mul(h_t[:, dt, :],
                                                 a_both[:, dt, :],
                                                 dbx[:, dt, :])
                        else:
                            nc.vector.tensor_tensor_scan(
                                h_t[:, dt, :], a_both[:, dt, :],
                                dbx[:, dt, :],
                                initial=carry[:, col:col + 1],
                                op0=OP.mult, op1=OP.add)
                            nc.vector.tensor_copy(carry[:, col:col + 1],
                                                  h_t[:, dt, lc - 1:lc])
                    w_t = npool.tile([128, DT2, lc], bf16, tag="w")
                    mul_to(pickmul(), w_t[:], h_t[:],
                           cc.broadcast_to([128, DT2, lc]))
                    for dt in range(DT2):
                        nc.tensor.matmul(y_ps[:, dt, :], eye16[:],
                                         w_t[:, dt, :],
                                         start=(n == 0),
                                         stop=(n == NSTATE - 1))

                # ---- y = y_ssm + D*xs, gate with silu(z) ----
                yg = wpool.tile([128, DT2, lc], bf16, tag="yg")
                for dt in range(DT2):
                    ys = spool.tile([128, lc], bf16, tag="ys")
                    nc.vector.scalar_tensor_tensor(
                        ys[:], xs_all[:, dt, t0:t0 + lc],
                        d_in_sb[:, dt:dt + 1],
                        y_ps[:, dt, :], op0=OP.mult, op1=OP.add)
                    (nc.gpsimd if small_pool else nc.vector).tensor_mul(
                        yg[:, dt, :], ys[:], z_all[:, dt, t0:t0 + lc])

                # ---- out_proj partial -> DRAM (bf16; host sums) ----
                for mt in range(8):
                    ps = psmm.tile([128, lc], f32, tag="mm")
                    for kt in range(DT2):
                        nc.tensor.matmul(
                            ps[:],
                            w_out_sb[:, kt, 128 * mt:128 * (mt + 1)],
                            yg[:, kt, :],
                            start=(kt == 0), stop=(kt == DT2 - 1))
                    ob = spool.tile([128, lc], bf16, tag="ob")
                    if pick2("ob", ob_act, 8) == 0:
                        nc.scalar.copy(ob[:], ps[:])
                    else:
                        nc.vector.tensor_copy(ob[:], ps[:])
                    nc.sync.dma_start(
                        y_part[128 * mt:128 * (mt + 1), t0:t0 + lc], ob[:])

            stages = {}
            for step in range(n_chunks + lag + 1):
                if step < n_chunks and "a" in passes:
                    emit_pass_a(step)
                if "b" not in passes:
                    continue
                bc = step - lag
                if 0 <= bc < n_chunks:
                    stages[bc] = emit_dt_stage(bc)
                if 1 <= bc <= n_chunks:
                    emit_scan_loop(stages.pop(bc - 1))

    nc.compile()
    return nc


_CACHED = {}


def _get_nc(**kw):
    key = tuple(sorted(kw.items()))
    if key not in _CACHED:
        _CACHED[key] = _build_nc(**kw)
    return _CACHED[key]


def _ladder_ok(inputs):
    # a(8+k) = a(k)*a(7) requires A[:, 8+k] == A[:, k] + A[:, 7]
    A = -np.exp(np.asarray(inputs["A_log"], np.float64))
    return bool(np.allclose(A[:, 8:16], A[:, 0:8] + A[:, 7:8], rtol=1e-5,
                            atol=1e-7))


def _host_prep(inputs):
    """Slice/transpose the full inputs into per-core in_maps."""
    import ml_dtypes
    _bf = ml_dtypes.bfloat16
    f32 = np.float32
    u = np.asarray(inputs["u"], f32)
    in_proj_w = np.asarray(inputs["in_proj_w"], f32)
    conv_w = np.asarray(inputs["conv_w"], f32)
    conv_b = np.asarray(inputs["conv_b"], f32)
    x_proj_w = np.asarray(inputs["x_proj_w"], f32)
    dt_proj_w = np.asarray(inputs["dt_proj_w"], f32)
    dt_bias = np.asarray(inputs["dt_bias"], f32)
    A_log = np.asarray(inputs["A_log"], f32)
    D_in = np.asarray(inputs["D_in"], f32)
    out_proj_w = np.asarray(inputs["out_proj_w"], f32)

    uT = np.ascontiguousarray(u.reshape(TOK, D_MODEL).T).astype(_bf)
    eye = np.eye(128, dtype=f32).astype(_bf)
    sel = np.kron(np.eye(2 * NSTATE, dtype=f32), np.ones((1, 128), f32)).astype(_bf)
    A = -np.exp(A_log)

    def fold(v):  # (256, k) -> (128, 2*k) with dtile-major columns
        v = v.reshape(DS, -1)
        return np.ascontiguousarray(
            np.concatenate([v[:128], v[128:]], axis=1))

    in_maps = []
    for k in range(NCORES):
        sl = slice(DS * k, DS * (k + 1))
        w_in_k = np.concatenate(
            [in_proj_w[sl], in_proj_w[D_INNER + DS * k:D_INNER + DS * (k + 1)]])
        in_maps.append({
            "uT": uT,
            "w_inT": np.ascontiguousarray(w_in_k.T).astype(_bf),
            "conv_w": fold(conv_w[sl]),
            "conv_b": fold(conv_b[sl]),
            "w_xpT": np.ascontiguousarray(x_proj_w[:, sl].T).astype(_bf),
            "w_dtT": np.ascontiguousarray(dt_proj_w[sl].T).astype(_bf),
            "dt_bias": fold(dt_bias[sl]),
            "a_neg": fold(A[sl]),
            "d_in": fold(D_in[sl]),
            "w_outT": np.ascontiguousarray(out_proj_w[:, sl].T).astype(_bf),
            "eye128": eye,
            "sel32": sel,
        })
    return in_maps


LAST_RESULTS = None


def bench(inputs, iters=24, warmup=4):
    """Estimate per-execution device time: device-put the sharded inputs
    once, then dispatch the jitted NEFF repeatedly (async) and time."""
    import time
    import jax
    from jax.sharding import Mesh, PartitionSpec, NamedSharding
    from jax.experimental.shard_map import shard_map
    import concourse.mybir as mybir
    from concourse import bass2jax
    from concourse.bass2jax import _bass_exec_p, install_neuronx_cc_hook

    install_neuronx_cc_hook()
    nc = _get_nc(ladder=_ladder_ok(inputs))
    in_maps = _host_prep(inputs)

    partition_name = (nc.partition_id_tensor.name
                      if nc.partition_id_tensor else None)
    in_names, out_names, out_avals, zero_outs = [], [], [], []
    for alloc in nc.m.functions[0].allocations:
        if not isinstance(alloc, mybir.MemoryLocationSet):
            continue
        name = alloc.memorylocations[0].name
        if alloc.kind == "ExternalInput":
            if name != partition_name:
                in_names.append(name)
        elif alloc.kind == "ExternalOutput":
            shape = tuple(alloc.tensor_shape)
            dtype = mybir.dt.np(alloc.dtype)
            out_avals.append(jax.core.ShapedArray(shape, dtype))
            out_names.append(name)
            zero_outs.append(np.zeros(shape, dtype))
    n_params = len(in_names)
    all_in_names = list(in_names) + list(out_names)
    if partition_name is not None:
        all_in_names.append(partition_name)

    def _body(*args):
        operands = list(args)
        if partition_name is not None:
            operands.append(bass2jax.partition_id_tensor())
        outs = _bass_exec_p.bind(
            *operands,
            out_avals=tuple(out_avals),
            in_names=tuple(all_in_names),
            out_names=tuple(out_names),
            lowering_input_output_aliases=(),
            sim_require_finite=True,
            sim_require_nnan=True,
            nc=nc,
        )
        return tuple(outs)

    devices = jax.devices()[:NCORES]
    mesh = Mesh(np.asarray(devices), ("core",))
    in_specs = (PartitionSpec("core"),) * (n_params + len(out_names))
    out_specs = (PartitionSpec("core"),) * len(out_names)
    fn = jax.jit(shard_map(_body, mesh=mesh, in_specs=in_specs,
                           out_specs=out_specs, check_rep=False),
                 keep_unused=True)

    concat_in = [np.concatenate([in_maps[c][nm] for c in range(NCORES)],
                                axis=0) for nm in in_names]
    concat_zeros = [np.zeros((NCORES * z.shape[0], *z.shape[1:]), z.dtype)
                    for z in zero_outs]
    sh = NamedSharding(mesh, PartitionSpec("core"))
    dev_in = [jax.device_put(a, sh) for a in concat_in + concat_zeros]

    for _ in range(warmup):
        outs = fn(*dev_in)
    jax.block_until_ready(outs)
    # median of several two-point marginals: robust to the occasional
    # multi-ms network hiccup of the axon proxy contaminating one batch.
    # Non-positive / wild estimates (hiccup in the small batch) are dropped.
    ests, upper = [], []
    for _ in range(7):
        times = {}
        for it in (max(1, iters // 8), iters // 2):
            t0 = time.perf_counter()
            for _ in range(it):
                outs = fn(*dev_in)
            jax.block_until_ready(outs)
            times[it] = time.perf_counter() - t0
        ks = sorted(times)
        ests.append((times[ks[1]] - times[ks[0]]) / (ks[1] - ks[0]))
        upper.append(times[ks[1]] / ks[1])
    good = sorted(e for e in ests if e > 0)
    if good:
        med = good[len(good) // 2]
        kept = [e for e in good if med / 2 <= e <= med * 2]
        if kept:
            return kept[len(kept) // 2]
    return min(upper)


def kernel(**inputs):
    global LAST_RESULTS
    from concourse import bass_utils

    u = np.asarray(inputs["u"], np.float32)
    D_skip = np.asarray(inputs["D_skip"], np.float32)

    nc = _get_nc(ladder=_ladder_ok(inputs))
    in_maps = _host_prep(inputs)
    res = bass_utils.run_bass_kernel_spmd(
        nc, in_maps, core_ids=list(range(NCORES)), trace=False)
    LAST_RESULTS = res

    acc = np.zeros((D_MODEL, TOK), np.float32)
    for r in res.results:
        acc += np.asarray(r["y_part"]).astype(np.float32)
    y = acc.T.reshape(BATCH, SEQ, D_MODEL)
    return y + D_skip[None, None, :] * u



# revision 22
# speedup vs baseline: 1.0271x; 1.0271x over previous
"""Mamba mixer (nn_Mixer) Trainium2 Bass kernel, v3.

Sharding: tensor-parallel over d_inner (2048 -> 256 per core, 8 cores).

Calibrated per-op HW costs (FD=1024 bf16): DVE copy 345ns, TT 616ns,
scan 2195ns (2.08 cyc/elem), STT 1213ns (1x); ACT 1054ns, table-set
switch +2.85us; Pool TT 2114ns. Design:
  - DVE: scans (only engine that can) + a share of TT muls
  - ACT: bc2 copies, silus, softplus, ladder squares (Square/Copy are in
    every ACT table set -> no switches in the scan era)
  - Pool: the remaining TT muls
  - Exp/Ln (natural_log_exp set) batched per dt_batch chunks
  - a-ladder: a_n = a0^(n+1); odd n = Square(a_(n-1)/2) on ACT; even n
    = a_(k-1)*a_k on DVE/Pool; a0 = Exp(-dt) in the batch region.
  - h carry chaining via persistent h-tile ring (no carry copies).

Self-contained: hardcodes all shapes.
"""

import numpy as np

D_MODEL = 1024
D_INNER = 2048
NSTATE = 16
DT_RANK = 64
DCONV = 4
BATCH = 2
SEQ = 4096

NCORES = 8
DS = D_INNER // NCORES          # 256 d_inner rows per core
DT2 = DS // 128                 # 2 partition tiles per core
TOK = BATCH * SEQ


def _build_nc(lc=512, fake_collective=False, n_ar=4, lag=5,
              dt_batch=4, mul_pool=20, bc_act=16, ob_act=8,
              skip_ar=False, passes="ab", ladder=True,
              ld_evens_tt=True, selpack=True, bcast_dma=False):
    """Build the Bass program (same SPMD program for all 8 cores).

    mul_pool: of 32 Bresenham slots for dbx/w/even-ladder TT muls, how
    many go to Pool (rest DVE).
    bc_act: of 16 bc2 copies per chunk, how many on ACT (rest DVE).
    ob_act: of 8 out-proj copies per chunk, how many on ACT (rest DVE).
    """
    import concourse.bass as bass
    import concourse.bacc as bacc
    import concourse.mybir as mybir
    import concourse.tile as tile

    f32 = mybir.dt.float32
    bf16 = mybir.dt.bfloat16
    AF = mybir.ActivationFunctionType
    OP = mybir.AluOpType

    n_chunks = TOK // lc
    chunks_per_b = SEQ // lc
    chunks_per_ar = n_chunks // n_ar
    lag = max(lag, chunks_per_ar)
    assert dt_batch <= chunks_per_ar and chunks_per_ar % dt_batch == 0

    nc = bacc.Bacc("TRN2", target_bir_lowering=False, debug=False,
                   num_devices=NCORES)

    # ---- kernel I/O (per-core shards prepared on the host) ----
    uT = nc.dram_tensor("uT", [D_MODEL, TOK], bf16, kind="ExternalInput")
    w_in = nc.dram_tensor("w_inT", [D_MODEL, 4 * 128], bf16, kind="ExternalInput")
    conv_w = nc.dram_tensor("conv_w", [128, DT2 * DCONV], f32, kind="ExternalInput")
    conv_b = nc.dram_tensor("conv_b", [128, DT2], f32, kind="ExternalInput")
    w_xp = nc.dram_tensor("w_xpT", [DS, DT_RANK + 2 * NSTATE], bf16, kind="ExternalInput")
    w_dt = nc.dram_tensor("w_dtT", [DT_RANK, DS], bf16, kind="ExternalInput")
    dt_bias = nc.dram_tensor("dt_bias", [128, DT2], f32, kind="ExternalInput")
    a_neg = nc.dram_tensor("a_neg", [128, DT2 * NSTATE], f32, kind="ExternalInput")
    d_in = nc.dram_tensor("d_in", [128, DT2], f32, kind="ExternalInput")
    w_out = nc.dram_tensor("w_outT", [DS, D_MODEL], bf16, kind="ExternalInput")
    eye_d = nc.dram_tensor("eye128", [128, 128], bf16, kind="ExternalInput")
    if selpack:
        # packed selectors: partition-group g (32 rows) slot i holds the
        # broadcast selector for index m = 4*i + g (m in [0, 32))
        sel_d = nc.dram_tensor("selp", [128, 8 * 128], bf16,
                               kind="ExternalInput")
    else:
        sel_d = nc.dram_tensor("sel32", [2 * NSTATE, 2 * NSTATE * 128],
                               bf16, kind="ExternalInput")
    y_part = nc.dram_tensor("y_part", [D_MODEL, TOK], bf16, kind="ExternalOutput")

    NXD = DT_RANK + 2 * NSTATE  # 96

    with tile.TileContext(nc) as tc:
        with (
            tc.tile_pool(name="const", bufs=1) as cpool,
            tc.tile_pool(name="u", bufs=2) as upool,
            tc.tile_pool(name="seq", bufs=lag + 3) as qpool,   # xs/sz rings
            tc.tile_pool(name="hring", bufs=17) as hpool,
            tc.tile_pool(name="aring", bufs=8) as apool,
            tc.tile_pool(name="dtb", bufs=dt_batch + 2) as dtpool,
            tc.tile_pool(name="work", bufs=2) as wpool,
            tc.tile_pool(name="nwork", bufs=3) as npool,
            tc.tile_pool(name="small", bufs=3) as spool,
            tc.tile_pool(name="mm", bufs=2, space="PSUM") as psmm,
            tc.tile_pool(name="psbc", bufs=2, space="PSUM") as psbc,
            tc.tile_pool(name="psy", bufs=1, space="PSUM") as psy,
            tc.tile_pool(name="dram", bufs=1, space="DRAM") as dpool,
        ):
            # ---- static weights into SBUF ----
            w_in_sb = cpool.tile([128, 8, 4 * 128], bf16)
            nc.sync.dma_start(w_in_sb[:], w_in.ap().rearrange(
                "(j p) m -> p j m", p=128))
            w_out_sb = cpool.tile([128, DT2, D_MODEL], bf16)
            nc.sync.dma_start(w_out_sb[:], w_out.ap().rearrange(
                "(k p) m -> p k m", p=128))
            w_xp_sb = cpool.tile([128, DT2, NXD], bf16)
            nc.sync.dma_start(w_xp_sb[:], w_xp.ap().rearrange(
                "(k p) m -> p k m", p=128))
            w_dt_sb = cpool.tile([DT_RANK, DS], bf16)
            nc.sync.dma_start(w_dt_sb[:], w_dt.ap())
            conv_w_sb = cpool.tile([128, DT2 * DCONV], f32)
            nc.sync.dma_start(conv_w_sb[:], conv_w.ap())
            conv_b_sb = cpool.tile([128, DT2], f32)
            nc.sync.dma_start(conv_b_sb[:], conv_b.ap())
            dt_bias_sb = cpool.tile([128, DT2], f32)
            nc.sync.dma_start(dt_bias_sb[:], dt_bias.ap())
            a_sb = cpool.tile([128, DT2 * NSTATE], f32)
            nc.sync.dma_start(a_sb[:], a_neg.ap())
            d_in_sb = cpool.tile([128, DT2], f32)
            nc.sync.dma_start(d_in_sb[:], d_in.ap())
            eye16 = cpool.tile([128, 128], bf16)
            nc.sync.dma_start(eye16[:], eye_d.ap())
            if selpack:
                sel_sb = cpool.tile([128, 8, 128], bf16)
                nc.sync.dma_start(sel_sb[:], sel_d.ap().rearrange(
                    "p (i m) -> p i m", m=128))
            else:
                sel_sb = cpool.tile([2 * NSTATE, 2 * NSTATE * 128], bf16)
                nc.sync.dma_start(sel_sb[:], sel_d.ap())

            # AllReduce segments (DRAM)
            seg_tok = TOK // n_ar
            ar_in = [dpool.tile([NXD, seg_tok], bf16, name=f"arin{s}",
                                tag=f"arin{s}")
                     for s in range(n_ar)]
            ar_out = [dpool.tile([NXD, seg_tok], bf16, name=f"arout{s}",
                                 tag=f"arout{s}", addr_space="Shared")
                      for s in range(n_ar)]

            uT_ap = uT.ap().rearrange("(j p) t -> p j t", p=128)

            cnt = {}

            def pick(key, num, den):
                c = cnt.get(key, 0)
                cnt[key] = c + 1
                return 0 if (c * num) % den < num else 1

            xs_ring, sz_ring, prev_h = {}, {}, {}

            # ================= Pass A =================
            prev_x = [None]

            def emit_pass_a(c):
                t0 = c * lc
                first_in_batch = (c % chunks_per_b) == 0
                u_sb = upool.tile([128, 8, lc], bf16, tag="u")
                nc.sync.dma_start(u_sb[:], uT_ap[:, :, t0:t0 + lc])

                xs_t = qpool.tile([128, DT2, lc], bf16, tag="xs")
                sz_t = qpool.tile([128, DT2, lc], bf16, tag="sz")
                xs_ring[c] = xs_t
                sz_ring[c] = sz_t
                xs_ring.pop(c - lag - 2, None)
                sz_ring.pop(c - lag - 2, None)

                x_sb = wpool.tile([128, DT2, lc + DCONV - 1], bf16, tag="x")
                for mt in range(4):
                    ps = psmm.tile([128, lc], f32, tag="mm")
                    for j in range(8):
                        nc.tensor.matmul(
                            ps[:],
                            w_in_sb[:, j, 128 * mt:128 * (mt + 1)],
                            u_sb[:, j, :],
                            start=(j == 0), stop=(j == 7))
                    if mt < DT2:
                        nc.scalar.copy(x_sb[:, mt, DCONV - 1:], ps[:])
                    else:
                        nc.scalar.activation(sz_t[:, mt - DT2, :],
                                             ps[:], AF.Silu, bias=0.0)

                # halo columns (previous chunk's last DCONV-1 raw x)
                for dt in range(DT2):
                    if first_in_batch:
                        nc.vector.memset(x_sb[:, dt, 0:DCONV - 1], 0.0)
                    else:
                        nc.vector.tensor_copy(
                            x_sb[:, dt, 0:DCONV - 1],
                            prev_x[0][:, dt, lc:lc + DCONV - 1])
                prev_x[0] = x_sb

                # causal conv1d (tensor_scalar taps + TT adds) + silu -> xs
                for dt in range(DT2):
                    ts = []
                    for k in range(DCONV):
                        tp = spool.tile([128, lc], bf16, tag="tp", bufs=4)
                        nc.vector.tensor_scalar_mul(
                            tp[:], x_sb[:, dt, k:k + lc],
                            conv_w_sb[:, dt * DCONV + k:dt * DCONV + k + 1])
                        ts.append(tp)
                    ca = spool.tile([128, lc], bf16, tag="ca")
                    cb = spool.tile([128, lc], bf16, tag="cb")
                    nc.vector.tensor_add(ca[:], ts[0][:], ts[1][:])
                    nc.vector.tensor_add(cb[:], ts[2][:], ts[3][:])
                    cs = spool.tile([128, lc], bf16, tag="cs")
                    nc.vector.tensor_add(cs[:], ca[:], cb[:])
                    nc.scalar.activation(xs_t[:, dt, :], cs[:],
                                         AF.Silu,
                                         bias=conv_b_sb[:, dt:dt + 1])

                # x_proj partial -> DRAM segment
                ps_xd = psmm.tile([NXD, lc], f32, tag="mm")
                for dt in range(DT2):
                    nc.tensor.matmul(ps_xd[:], w_xp_sb[:, dt, :],
                                     xs_t[:, dt, :],
                                     start=(dt == 0), stop=(dt == DT2 - 1))
                xd_loc = spool.tile([NXD, lc], bf16, tag="xdloc")
                nc.scalar.copy(xd_loc[:], ps_xd[:])
                seg = c // chunks_per_ar
                s0 = t0 - seg * seg_tok
                nc.sync.dma_start(ar_in[seg][:, s0:s0 + lc], xd_loc[:])

                if (c + 1) % chunks_per_ar == 0:
                    if skip_ar:
                        pass
                    elif fake_collective:
                        nc.sync.dma_start(ar_out[seg][:], ar_in[seg][:])
                    else:
                        nc.gpsimd.collective_compute(
                            "AllReduce", OP.add,
                            replica_groups=[list(range(NCORES))],
                            ins=[ar_in[seg].opt()], outs=[ar_out[seg].opt()])

            # ============ dt batch (Exp/Ln table-set region) ============
            def emit_dt_batch(c0):
                out = {}
                for c in range(c0, c0 + dt_batch):
                    t0 = c * lc
                    seg = c // chunks_per_ar
                    s0 = t0 - seg * seg_tok
                    xdt_sb = spool.tile([DT_RANK, lc], bf16, tag="xdt")
                    nc.sync.dma_start(xdt_sb[:],
                                      ar_out[seg][0:DT_RANK, s0:s0 + lc])
                    dt_sb = dtpool.tile([128, DT2, lc], bf16, tag="dt")
                    a0_sb = dtpool.tile([128, DT2, lc], bf16, tag="a0")
                    for dt in range(DT2):
                        ps = psmm.tile([128, lc], f32, tag="mm")
                        nc.tensor.matmul(
                            ps[:], w_dt_sb[:, 128 * dt:128 * (dt + 1)],
                            xdt_sb[:], start=True, stop=True)
                        e_t = spool.tile([128, lc], bf16, tag="esp")
                        nc.scalar.activation(e_t[:], ps[:], AF.Exp,
                                             bias=dt_bias_sb[:, dt:dt + 1])
                        nc.scalar.activation(dt_sb[:, dt, :], e_t[:], AF.Ln,
                                             bias=1.0)
                    # a0 = exp(-dt) over both dt tiles at once
                    nc.scalar.activation(a0_sb[:], dt_sb[:], AF.Exp,
                                         bias=0.0, scale=-1.0)
                    out[c] = (dt_sb, a0_sb)
                return out

            # ================= Pass B scan era =================
            def emit_scan_chunk(c, dt_sb, a0_sb):
                t0 = c * lc
                seg = c // chunks_per_ar
                s0 = t0 - seg * seg_tok
                first_in_batch = (c % chunks_per_b) == 0
                xs_t = xs_ring[c]
                sz_t = sz_ring[c]

                # B/C rows of x_dbl, replicated onto all 4 partition groups
                bc32 = spool.tile([128, lc], bf16, tag="bc32")
                if bcast_dma:
                    nc.sync.dma_start(
                        bc32[:].rearrange("(g p) t -> g p t", p=2 * NSTATE),
                        ar_out[seg][DT_RANK:NXD, s0:s0 + lc]
                        .unsqueeze(0).broadcast_to([4, 2 * NSTATE, lc]))
                else:
                    for g in range(4 if selpack else 1):
                        nc.sync.dma_start(
                            bc32[32 * g:32 * (g + 1), :],
                            ar_out[seg][DT_RANK:NXD, s0:s0 + lc])

                dtx_sb = wpool.tile([128, DT2, lc], bf16, tag="dtx")
                nc.vector.tensor_mul(dtx_sb[:], dt_sb[:], xs_t[:])

                a_tiles = {}

                def mul_tt(out, in0, in1):
                    if pick("mul", mul_pool, 32) == 0:
                        nc.gpsimd.tensor_mul(out, in0, in1)
                    else:
                        nc.vector.tensor_mul(out, in0, in1)

                y_ps = psy.tile([128, DT2, lc], f32, tag="y")
                h_prev = prev_h.get(c - 1) if not first_in_batch else None
                h_cur = {}
                for n in range(NSTATE):
                    bc_ps = psbc.tile([128, 2, lc], f32, tag="bcps")
                    if selpack:
                        # selector m: group g = m % 4, slot i = m // 4
                        for half, m in ((0, n), (1, NSTATE + n)):
                            g, i = m % 4, m // 4
                            nc.tensor.matmul(
                                bc_ps[:, half, :],
                                sel_sb[32 * g:32 * (g + 1), i, :],
                                bc32[32 * g:32 * (g + 1), :],
                                start=True, stop=True,
                                tile_position=(32 * g, 0))
                    else:
                        for half, m in ((0, n), (1, NSTATE + n)):
                            nc.tensor.matmul(
                                bc_ps[:, half, :],
                                sel_sb[:, 128 * m:128 * (m + 1)],
                                bc32[0:2 * NSTATE, :],
                                start=True, stop=True)
                    bc2 = npool.tile([128, 2, lc], bf16, tag="bc2")
                    if pick("bc2", bc_act, 16) == 0:
                        nc.scalar.copy(bc2[:], bc_ps[:])
                    else:
                        nc.vector.tensor_copy(bc2[:], bc_ps[:])
                    bb = bc2[:, 0:1, :]
                    cc = bc2[:, 1:2, :]

                    # ---- a_n = a0^(n+1) ----
                    if n == 0:
                        a_n = a0_sb
                    elif ladder and n % 2 == 1:
                        k = (n - 1) // 2
                        a_n = apool.tile([128, DT2, lc], bf16, tag="a")
                        nc.scalar.activation(a_n[:], a_tiles[k][:],
                                             AF.Square, bias=0.0)
                    elif ladder and ld_evens_tt:
                        k = n // 2
                        a_n = apool.tile([128, DT2, lc], bf16, tag="a")
                        mul_tt(a_n[:], a_tiles[k - 1][:], a_tiles[k][:])
                    else:
                        a_n = apool.tile([128, DT2, lc], bf16, tag="a")
                        for dt in range(DT2):
                            col = dt * NSTATE + n
                            nc.scalar.activation(
                                a_n[:, dt, :], dt_sb[:, dt, :], AF.Exp,
                                bias=0.0, scale=a_sb[:, col:col + 1])
                    a_tiles[n] = a_n

                    dbx = npool.tile([128, DT2, lc], bf16, tag="dbx")
                    mul_tt(dbx[:], dtx_sb[:],
                           bb.broadcast_to([128, DT2, lc]))
                    h_t = hpool.tile([128, DT2, lc], bf16, tag="h")
                    for dt in range(DT2):
                        if h_prev is None:
                            init = 0.0
                        else:
                            init = h_prev[n][:, dt, lc - 1:lc]
                        nc.vector.tensor_tensor_scan(
                            h_t[:, dt, :], a_n[:, dt, :], dbx[:, dt, :],
                            initial=init, op0=OP.mult, op1=OP.add)
                    h_cur[n] = h_t
                    w_t = npool.tile([128, DT2, lc], bf16, tag="w")
                    mul_tt(w_t[:], h_t[:],
                           cc.broadcast_to([128, DT2, lc]))
                    for dt in range(DT2):
                        nc.tensor.matmul(y_ps[:, dt, :], eye16[:],
                                         w_t[:, dt, :],
                                         start=(n == 0),
                                         stop=(n == NSTATE - 1))
                prev_h[c] = h_cur
                prev_h.pop(c - 1, None)

                # ---- y = y_ssm + D*xs, gate with silu(z) ----
                yg = wpool.tile([128, DT2, lc], bf16, tag="yg")
                for dt in range(DT2):
                    ys = spool.tile([128, lc], bf16, tag="ys")
                    nc.vector.scalar_tensor_tensor(
                        ys[:], xs_t[:, dt, :],
                        d_in_sb[:, dt:dt + 1],
                        y_ps[:, dt, :], op0=OP.mult, op1=OP.add)
                    nc.vector.tensor_mul(
                        yg[:, dt, :], ys[:], sz_t[:, dt, :])

                # ---- out_proj partial -> DRAM (bf16; host sums) ----
                for mt in range(8):
                    ps = psmm.tile([128, lc], f32, tag="mm")
                    for kt in range(DT2):
                        nc.tensor.matmul(
                            ps[:],
                            w_out_sb[:, kt, 128 * mt:128 * (mt + 1)],
                            yg[:, kt, :],
                            start=(kt == 0), stop=(kt == DT2 - 1))
                    ob = spool.tile([128, lc], bf16, tag="ob")
                    if pick("ob", ob_act, 8) == 0:
                        nc.scalar.copy(ob[:], ps[:])
                    else:
                        nc.vector.tensor_copy(ob[:], ps[:])
                    nc.sync.dma_start(
                        y_part[128 * mt:128 * (mt + 1), t0:t0 + lc], ob[:])

            stages = {}
            for step in range(n_chunks + lag + 1):
                if step < n_chunks and "a" in passes:
                    emit_pass_a(step)
                if "b" not in passes:
                    continue
                bc = step - lag
                if 0 <= bc < n_chunks and bc % dt_batch == 0:
                    stages.update(emit_dt_batch(bc))
                if 1 <= bc <= n_chunks and (bc - 1) in stages:
                    dt_sb, a0_sb = stages.pop(bc - 1)
                    emit_scan_chunk(bc - 1, dt_sb, a0_sb)

    nc.compile()
    return nc


_CACHED = {}


def _get_nc(**kw):
    key = tuple(sorted(kw.items()))
    if key not in _CACHED:
        _CACHED[key] = _build_nc(**kw)
    return _CACHED[key]


def _ladder_ok(inputs):
    # a_n = a0^(n+1) requires A[:, n] == (n+1) * A[:, 0]
    A = -np.exp(np.asarray(inputs["A_log"], np.float64))
    mult = np.arange(1, NSTATE + 1)[None, :]
    return bool(np.allclose(A, A[:, 0:1] * mult, rtol=1e-5, atol=1e-7))


def _host_prep(inputs):
    """Slice/transpose the full inputs into per-core in_maps."""
    import ml_dtypes
    _bf = ml_dtypes.bfloat16
    f32 = np.float32
    u = np.asarray(inputs["u"], f32)
    in_proj_w = np.asarray(inputs["in_proj_w"], f32)
    conv_w = np.asarray(inputs["conv_w"], f32)
    conv_b = np.asarray(inputs["conv_b"], f32)
    x_proj_w = np.asarray(inputs["x_proj_w"], f32)
    dt_proj_w = np.asarray(inputs["dt_proj_w"], f32)
    dt_bias = np.asarray(inputs["dt_bias"], f32)
    A_log = np.asarray(inputs["A_log"], f32)
    D_in = np.asarray(inputs["D_in"], f32)
    out_proj_w = np.asarray(inputs["out_proj_w"], f32)

    uT = np.ascontiguousarray(u.reshape(TOK, D_MODEL).T).astype(_bf)
    eye = np.eye(128, dtype=f32).astype(_bf)
    # packed selectors: selector m lives at partition group g=m%4, slot
    # i=m//4, nonzero row = bc-row m within the group's 32-row replica
    selp = np.zeros((128, 8, 128), f32)
    for m in range(2 * NSTATE):
        g, i = m % 4, m // 4
        selp[32 * g + m, i, :] = 1.0
    selp = selp.reshape(128, 8 * 128).astype(_bf)
    sel32 = np.kron(np.eye(2 * NSTATE, dtype=f32),
                    np.ones((1, 128), f32)).astype(_bf)
    A = -np.exp(A_log)

    def fold(v):  # (256, k) -> (128, 2*k) with dtile-major columns
        v = v.reshape(DS, -1)
        return np.ascontiguousarray(
            np.concatenate([v[:128], v[128:]], axis=1))

    in_maps = []
    for k in range(NCORES):
        sl = slice(DS * k, DS * (k + 1))
        w_in_k = np.concatenate(
            [in_proj_w[sl], in_proj_w[D_INNER + DS * k:D_INNER + DS * (k + 1)]])
        in_maps.append({
            "uT": uT,
            "w_inT": np.ascontiguousarray(w_in_k.T).astype(_bf),
            "conv_w": fold(conv_w[sl]),
            "conv_b": fold(conv_b[sl]),
            "w_xpT": np.ascontiguousarray(x_proj_w[:, sl].T).astype(_bf),
            "w_dtT": np.ascontiguousarray(dt_proj_w[sl].T).astype(_bf),
            "dt_bias": fold(dt_bias[sl]),
            "a_neg": fold(A[sl]),
            "d_in": fold(D_in[sl]),
            "w_outT": np.ascontiguousarray(out_proj_w[:, sl].T).astype(_bf),
            "eye128": eye,
            "selp": selp,
            "sel32": sel32,
        })
    return in_maps


LAST_RESULTS = None


def bench(inputs, iters=24, warmup=4, **build_kw):
    """Estimate per-execution device time: device-put the sharded inputs
    once, then dispatch the jitted NEFF repeatedly (async) and time."""
    import time
    import jax
    from jax.sharding import Mesh, PartitionSpec, NamedSharding
    from jax.experimental.shard_map import shard_map
    import concourse.mybir as mybir
    from concourse import bass2jax
    from concourse.bass2jax import _bass_exec_p, install_neuronx_cc_hook

    install_neuronx_cc_hook()
    nc = _get_nc(ladder=_ladder_ok(inputs), **build_kw)
    in_maps = _host_prep(inputs)

    partition_name = (nc.partition_id_tensor.name
                      if nc.partition_id_tensor else None)
    in_names, out_names, out_avals, zero_outs = [], [], [], []
    for alloc in nc.m.functions[0].allocations:
        if not isinstance(alloc, mybir.MemoryLocationSet):
            continue
        name = alloc.memorylocations[0].name
        if alloc.kind == "ExternalInput":
            if name != partition_name:
                in_names.append(name)
        elif alloc.kind == "ExternalOutput":
            shape = tuple(alloc.tensor_shape)
            dtype = mybir.dt.np(alloc.dtype)
            out_avals.append(jax.core.ShapedArray(shape, dtype))
            out_names.append(name)
            zero_outs.append(np.zeros(shape, dtype))
    n_params = len(in_names)
    all_in_names = list(in_names) + list(out_names)
    if partition_name is not None:
        all_in_names.append(partition_name)

    def _body(*args):
        operands = list(args)
        if partition_name is not None:
            operands.append(bass2jax.partition_id_tensor())
        outs = _bass_exec_p.bind(
            *operands,
            out_avals=tuple(out_avals),
            in_names=tuple(all_in_names),
            out_names=tuple(out_names),
            lowering_input_output_aliases=(),
            sim_require_finite=True,
            sim_require_nnan=True,
            nc=nc,
        )
        return tuple(outs)

    devices = jax.devices()[:NCORES]
    mesh = Mesh(np.asarray(devices), ("core",))
    in_specs = (PartitionSpec("core"),) * (n_params + len(out_names))
    out_specs = (PartitionSpec("core"),) * len(out_names)
    fn = jax.jit(shard_map(_body, mesh=mesh, in_specs=in_specs,
                           out_specs=out_specs, check_rep=False),
                 keep_unused=True)

    concat_in = [np.concatenate([in_maps[c][nm] for c in range(NCORES)],
                                axis=0) for nm in in_names]
    concat_zeros = [np.zeros((NCORES * z.shape[0], *z.shape[1:]), z.dtype)
                    for z in zero_outs]
    sh = NamedSharding(mesh, PartitionSpec("core"))
    dev_in = [jax.device_put(a, sh) for a in concat_in + concat_zeros]

    for _ in range(warmup):
        outs = fn(*dev_in)
    jax.block_until_ready(outs)
    ests, upper = [], []
    for _ in range(7):
        times = {}
        for it in (max(1, iters // 8), iters // 2):
            t0 = time.perf_counter()
            for _ in range(it):
                outs = fn(*dev_in)
            jax.block_until_ready(outs)
            times[it] = time.perf_counter() - t0
        ks = sorted(times)
        ests.append((times[ks[1]] - times[ks[0]]) / (ks[1] - ks[0]))
        upper.append(times[ks[1]] / ks[1])
    good = sorted(e for e in ests if e > 0)
    if good:
        med = good[len(good) // 2]
        kept = [e for e in good if med / 2 <= e <= med * 2]
        if kept:
            return kept[len(kept) // 2]
    return min(upper)


def kernel(**inputs):
    global LAST_RESULTS
    from concourse import bass_utils

    u = np.asarray(inputs["u"], np.float32)
    D_skip = np.asarray(inputs["D_skip"], np.float32)

    nc = _get_nc(ladder=_ladder_ok(inputs))
    in_maps = _host_prep(inputs)
    res = bass_utils.run_bass_kernel_spmd(
        nc, in_maps, core_ids=list(range(NCORES)), trace=False)
    LAST_RESULTS = res

    acc = np.zeros((D_MODEL, TOK), np.float32)
    for r in res.results:
        acc += np.asarray(r["y_part"]).astype(np.float32)
    y = acc.T.reshape(BATCH, SEQ, D_MODEL)
    return y + D_skip[None, None, :] * u
